# revision 31
# baseline (speedup 1.0000x reference)
"""Trainium2 Bass kernel for nn_MultiHeadAttention_48395691492077.

Reference (B=4, S=2048, D=1024, single head, anti-causal triu mask):
    qkv = x @ wqkv; q,k,v = split(qkv)
    scores = triu(q @ k^T / sqrt(B));  masked softmax over keys t >= s
    x2  = softmax(scores) @ v @ w_lin + b_lin + x
    out = relu(x2 @ w_ff1 + b_ff1) @ w_ff2 + b_ff2 + x2

Sharding: 8 cores = 4 batches x 2 query-halves. Each core computes the
full-batch key/value projections (duplicated within a pair) and attention
for its own 1024 queries. The program is identical on all cores (SPMD);
per-core differences (which queries, which mask pattern) are carried in
the input data.

Device algebra (everything transposed so no on-chip transposes are needed):
    zT = wz^T.T @ xT   with wz = (Wk @ Wq^T)/2  (host-precomputed)
    scoresT[t,s] = sum_a zT[a,t] * xT[a,s]
    expT = exp(scoresT) * mask01               (no max-subtraction; |s|<=36)
    den[s] broadcast = ones[128,128].T @ expT (PE), rbs = 1/den (DVE)
    numerator^T[d,s] = v[t,d].T @ expT;  attn^T = numerator^T * rbs
    x2T = w_lin.T @ attn^T + (xT + b_lin);  hT = relu(w_ff1.T @ x2T + b_ff1)
    outT = w_ff2.T @ hT + x2T               (+ b_ff2 added on host)
Matmul inputs are bf16 (fp32 PSUM accumulation); residuals are fp32.
"""

import numpy as np
import ml_dtypes

B, S, D = 4, 2048, 1024
NCORES = 8
BF16 = ml_dtypes.bfloat16
E4M3 = ml_dtypes.float8_e4m3
FF_SCALE = 64.0  # fp8 weight prescale (keeps 0.02-sigma weights normal)

NT = S // 128            # 16 t-chunks
ND = D // 128            # 8 chunks of 128 along any D-sized dim

# global query-column starts of (sb0, sb1) per parity
SB_GLOBAL = {0: (0, 1536), 1: (512, 1024)}
# t-chunks each (parity, s-block) actually needs (branch-specialized)
SB_SLOTS = {
    0: {0: list(range(0, NT)), 1: list(range(12, NT))},
    1: {0: list(range(4, NT)), 1: list(range(8, NT))},
}


_COMPILED = None
_LAST_IN_MAPS = None


def _mask_order(parity: int):
    return [(sb, tc) for sb in (0, 1) for tc in SB_SLOTS[parity][sb]]


def _build_masks(parity: int) -> np.ndarray:
    """[20, 128, 512] bf16 multiplicative masks, one per processed block."""
    order = _mask_order(parity)
    m = np.zeros((len(order), 128, 512), np.float32)
    ii = np.arange(128)[:, None]
    jj = np.arange(512)[None, :]
    for k, (sb, tc) in enumerate(order):
        s0 = SB_GLOBAL[parity][sb]
        m[k] = ((128 * tc + ii) >= (s0 + jj)).astype(np.float32)
    return m.astype(BF16)


def _build_program():
    from contextlib import ExitStack
    import concourse.bacc as bacc
    import concourse.mybir as mybir
    import concourse.tile as tile

    f32 = mybir.dt.float32
    b16 = mybir.dt.bfloat16
    f8 = mybir.dt.float8e4
    AF = mybir.ActivationFunctionType
    DR = mybir.MatmulPerfMode.DoubleRow

    nc = bacc.Bacc("TRN2", target_bir_lowering=False, debug=False,
                   num_devices=NCORES)

    # xT/qxT/wz/wv arrive partition-major ([128, chunk, cols] flattened) so
    # every DMA is one contiguous multi-KB segment per partition
    xT_d = nc.dram_tensor("xT", [128, ND * S], b16, kind="ExternalInput")
    qxT_d = nc.dram_tensor("qxT", [128, ND * 1024], b16, kind="ExternalInput")
    xq_d = nc.dram_tensor("xq", [128, ND * 1024], f32, kind="ExternalInput")
    wz_d = nc.dram_tensor("wz", [128, ND * D], b16, kind="ExternalInput")
    wv_d = nc.dram_tensor("wv", [128, ND * D], b16, kind="ExternalInput")
    wlin_d = nc.dram_tensor("wlin", [D, D], b16, kind="ExternalInput")
    # FFN weights in fp8 DoubleRow layout [128, pair, 2, D] (prescaled)
    wff1_d = nc.dram_tensor("wff1", [128, ND * D], f8, kind="ExternalInput")
    wff2_d = nc.dram_tensor("wff2", [128, ND * D], f8, kind="ExternalInput")
    masks_d = nc.dram_tensor("masks", [20, 128, 512], b16, kind="ExternalInput")
    par_d = nc.dram_tensor("par", [1, 1], mybir.dt.uint32, kind="ExternalInput")
    bf1_d = nc.dram_tensor("bf1", [128, ND], f32, kind="ExternalInput")
    outT_d = nc.dram_tensor("outT", [D, 1024], f32, kind="ExternalOutput")

    with tile.TileContext(nc) as tc:
        es = ExitStack()
        with es:
            pp = es.enter_context(tc.tile_pool(name="persist", bufs=1))
            sp = es.enter_context(tc.tile_pool(name="stream", bufs=2))
            ps = es.enter_context(
                tc.tile_pool(name="ps", bufs=8, space="PSUM"))
            esB = es.enter_context(ExitStack())
            pb = esB.enter_context(tc.tile_pool(name="pB", bufs=1))
            esA = ExitStack()
            pa = esA.enter_context(tc.tile_pool(name="pA", bufs=1,
                                                side="right"))

            def psum():
                t = ps.tile([128, 512], f32, tag="mm", bufs=8, name="mmps")
                return t

            # ---- constants ----
            ones_sq = pp.tile([128, 128], b16, tag="ones_sq", bufs=1)
            nc.vector.memset(ones_sq[:], 1.0)
            # warm the PE HAM clock-gate while input DMAs are in flight
            wups = psum()
            NWARM = 40
            for i in range(NWARM):
                nc.tensor.matmul(wups[:, 0:128], ones_sq[:], ones_sq[:],
                                 start=(i == 0), stop=(i == NWARM - 1))

            # ---- input loads ----
            wv_a = pa.tile([128, ND, D], b16, tag="wv", bufs=1)
            xt_a = pa.tile([128, ND, S], b16, tag="xt", bufs=1)

            def xt_dma(eng, d, h):
                eng.dma_start(
                    xt_a[:, d, h * 1024:(h + 1) * 1024],
                    xT_d.ap()[:, d * S + h * 1024:d * S + (h + 1) * 1024])

            def wv_dma(eng, d, h):
                eng.dma_start(wv_a[:, d, h * 512:(h + 1) * 512],
                              wv_d.ap()[:, d * D + h * 512:
                                        d * D + (h + 1) * 512])

            # d-chunk supply races the d-major v loop below: per-d need is
            # wv half (.125MB) + xt t-half (.25MB) per 1.7us of compute.
            # sync starts immediately and feeds d=0..3, scalar (delayed by
            # its act-table preamble) d=4..7; the late-needed wv vb=1 halves
            # ride the slow gpsimd queue
            for d in range(5):
                wv_dma(nc.sync, d, 0)
                xt_dma(nc.sync, d, 0)
            for d in range(5, ND):
                wv_dma(nc.gpsimd, d, 0)
                xt_dma(nc.scalar, d, 0)
            for d in range(ND):
                wv_dma(nc.gpsimd, d, 1)
            for d in range(5):
                xt_dma(nc.sync, d, 1)
            for d in range(5, ND):
                xt_dma(nc.scalar, d, 1)
            wz_a = pa.tile([128, ND, D], b16, tag="wz", bufs=1)
            nc.sync.dma_start(
                wz_a[:], wz_d.ap().rearrange("p (c n) -> p c n", c=ND))
            qx_a = pb.tile([128, ND, 1024], b16, tag="qx", bufs=1)
            nc.scalar.dma_start(
                qx_a[:], qxT_d.ap().rearrange("p (c n) -> p c n", c=ND))
            bf1_t = pp.tile([128, ND], f32, tag="bf1", bufs=1)
            nc.sync.dma_start(bf1_t[:], bf1_d.ap())
            wv_t = [wv_a[:, d] for d in range(ND)]
            xt = [xt_a[:, d] for d in range(ND)]
            wz_t = [wz_a[:, d] for d in range(ND)]
            qx = [qx_a[:, d] for d in range(ND)]

            # ---- phase A: v [t,d] then zT [a,t] projections ----
            zt = [pb.tile([128, S], b16, tag=f"zt{m}", bufs=1, name=f"zt{m}")
                  for m in range(ND)]
            vt = [pb.tile([128, D], b16, tag=f"vt{t}", bufs=1, name=f"vt{t}")
                  for t in range(NT)]

            # quadrant (vb, t-half): 8 live psums, contraction (d) outermost
            # so the first quadrant consumes xt/wv chunk d as it lands
            for vb in range(2):
                for th in range(2):
                    trange = range(th * 8, th * 8 + 8)
                    vps = {t: psum() for t in trange}
                    for d in range(ND):
                        for t in trange:
                            nc.tensor.matmul(
                                vps[t][:],
                                xt[d][:, t * 128:(t + 1) * 128],
                                wv_t[d][:, vb * 512:(vb + 1) * 512],
                                start=(d == 0), stop=(d == ND - 1))
                    for t in trange:
                        nc.vector.tensor_copy(
                            vt[t][:, vb * 512:(vb + 1) * 512], vps[t][:])

            for m in range(ND):
                zps = [psum() for _ in range(4)]
                for d in range(ND):
                    for cb in range(4):
                        nc.tensor.matmul(
                            zps[cb][:],
                            wz_t[d][:, m * 128:(m + 1) * 128],
                            xt[d][:, cb * 512:(cb + 1) * 512],
                            start=(d == 0), stop=(d == ND - 1))
                for cb in range(4):
                    nc.vector.tensor_copy(
                        zt[m][:, cb * 512:(cb + 1) * 512], zps[cb][:])

            # ---- free phase-A inputs; right pool for attn + phase-C weights
            esA.close()
            pr = es.enter_context(tc.tile_pool(name="pAC", bufs=1,
                                               side="right"))
            wl_a = pr.tile([128, ND, D], b16, tag="wl", bufs=1)
            nc.sync.dma_start(wl_a[:], wlin_d.ap()
                              .rearrange("(c p) n -> p c n", p=128))
            wlin_t = [wl_a[:, d] for d in range(ND)]
            wf18_a = pr.tile([128, 4, 2, D], f8, tag="wf1", bufs=1)
            nc.sync.dma_start(
                wf18_a[:],
                wff1_d.ap().rearrange("p (a b n) -> p a b n", a=4, b=2))
            wf28_a = pr.tile([128, 4, 2, D], f8, tag="wf2", bufs=1)
            nc.sync.dma_start(
                wf28_a[:],
                wff2_d.ap().rearrange("p (a b n) -> p a b n", a=4, b=2))
            # prefetch the full residual (4MB f32) on the scalar queue during
            # phase B so phase C's out-DMAs get the sync queue to themselves
            xqf = pr.tile([128, ND, 1024], f32, tag="xqf", bufs=1)
            nc.scalar.dma_start(
                xqf[:], xq_d.ap().rearrange("p (c n) -> p c n", c=ND))

            attn = [pr.tile([128, 1024], b16, tag=f"at{d}", bufs=1,
                            name=f"at{d}") for d in range(ND)]

            def phase_b(parity):
                sb_slots = SB_SLOTS[parity]
                order = _mask_order(parity)
                sb0s = SB_GLOBAL[parity]

                def wclip(sb, tcn):
                    # query cols of block (sb,tcn) with any unmasked t form
                    # the prefix [0:w); cols beyond never touch PE or DVE
                    return min(512, 128 * tcn + 128 - sb0s[sb])

                # pass 1: scoresT -> exp -> mask, tcn descending so pass 2
                # (also descending: its start=True block must be full-width)
                # never waits on the tail of pass 1
                et = {}
                for tcn in reversed(range(NT)):
                    sbs = [sb for sb in (0, 1) if tcn in sb_slots[sb]]
                    scp = {sb: psum() for sb in sbs}
                    for a in range(ND):
                        for sb in sbs:
                            w = wclip(sb, tcn)
                            nc.tensor.matmul(
                                scp[sb][:, 0:w],
                                zt[a][:, tcn * 128:(tcn + 1) * 128],
                                qx[a][:, sb * 512:sb * 512 + w],
                                start=(a == 0), stop=(a == ND - 1))
                    for sb in sbs:
                        w = wclip(sb, tcn)
                        e = pb.tile([128, 512], b16, tag=f"et{sb}_{tcn}",
                                    bufs=1, name=f"et{parity}_{sb}_{tcn}")
                        et[(sb, tcn)] = e
                        nc.scalar.activation(e[:, 0:w], scp[sb][:, 0:w],
                                             AF.Exp)
                        if 128 * tcn - sb0s[sb] < 512:  # block not all-ones
                            kidx = order.index((sb, tcn))
                            mk = sp.tile([128, 512], b16, tag="mks", bufs=4,
                                         name=f"mk{parity}_{kidx}")
                            nc.sync.dma_start(mk[:, 0:w],
                                              masks_d.ap()[kidx][:, 0:w])
                            nc.vector.tensor_mul(e[:, 0:w], e[:, 0:w],
                                                 mk[:, 0:w])

                # pass 2: den (broadcast), recip, AV, normalize
                rbs = {}
                for sb in (0, 1):
                    slots = sorted(sb_slots[sb], reverse=True)
                    den_ps = psum()
                    for k, tcn in enumerate(slots):
                        w = wclip(sb, tcn)
                        nc.tensor.matmul(
                            den_ps[:, 0:w], ones_sq[:],
                            et[(sb, tcn)][:, 0:w],
                            start=(k == 0), stop=(k == len(slots) - 1))
                    r = sp.tile([128, 512], f32, tag="rbs", bufs=2,
                                name=f"rbs{parity}_{sb}")
                    nc.vector.reciprocal(r[:], den_ps[:])
                    rbs[sb] = r

                for dc in range(ND):
                    avp = {sb: psum() for sb in (0, 1)}
                    for sb in (0, 1):
                        slots = sorted(sb_slots[sb], reverse=True)
                        for k, tcn in enumerate(slots):
                            w = wclip(sb, tcn)
                            nc.tensor.matmul(
                                avp[sb][:, 0:w],
                                vt[tcn][:, dc * 128:(dc + 1) * 128],
                                et[(sb, tcn)][:, 0:w],
                                start=(k == 0), stop=(k == len(slots) - 1))
                    for sb in (0, 1):
                        nc.vector.tensor_mul(
                            attn[dc][:, sb * 512:(sb + 1) * 512],
                            avp[sb][:], rbs[sb][:])

            par_regs = nc.alloc_registers("par_regs")
            nc.regs_load(par_regs, par_d.ap()[0:1, 0:1])
            par = nc.snap(par_regs, donate=True, min_val=0, max_val=1)
            with tc.If(par < 1) as cmp:
                phase_b(0)
            with cmp.Else():
                phase_b(1)

            # ---- free pB (zt/vt/qx/et); left pool for phase-C tiles ----
            esB.close()
            esC = es.enter_context(ExitStack())
            pc = esC.enter_context(tc.tile_pool(name="pC", bufs=1))

            x2s = [pc.tile([128, 1024], f32, tag=f"x2s{d}", bufs=1,
                           name=f"x2s{d}") for d in range(ND)]
            x28 = [pc.tile([128, 2, 1024], f8, tag=f"x28{p}", bufs=1,
                           name=f"x28{p}") for p in range(4)]
            ht8 = [pc.tile([128, 2, 1024], f8, tag=f"ht8{p}", bufs=1,
                           name=f"ht8{p}") for p in range(4)]

            # phase C per s2-half: the first half's outputs start draining
            # while the second half computes (out-DMA would otherwise bound
            # the kernel tail)
            for s2 in range(2):
                cc = slice(s2 * 512, (s2 + 1) * 512)
                for oc in range(ND):
                    cps = psum()
                    for d in range(ND):
                        nc.tensor.matmul(
                            cps[:],
                            wlin_t[d][:, oc * 128:(oc + 1) * 128],
                            attn[d][:, cc],
                            start=(d == 0), stop=(d == ND - 1))
                    nc.vector.tensor_add(x2s[oc][:, cc], cps[:],
                                         xqf[:, oc, cc])
                    nc.scalar.activation(x28[oc // 2][:, oc % 2, cc],
                                         x2s[oc][:, cc], AF.Copy,
                                         scale=1.0 / FF_SCALE)

                for fc in range(ND):
                    cps = psum()
                    for p in range(4):
                        nc.tensor.matmul(
                            cps[:],
                            wf18_a[:, p, :, fc * 128:(fc + 1) * 128],
                            x28[p][:, :, cc],
                            start=(p == 0), stop=(p == 3), perf_mode=DR)
                    nc.scalar.activation(ht8[fc // 2][:, fc % 2, cc], cps[:],
                                         AF.Relu, bias=bf1_t[:, fc:fc + 1],
                                         scale=1.0 / FF_SCALE)

                for oc in range(ND):
                    cps = psum()
                    for p in range(4):
                        nc.tensor.matmul(
                            cps[:],
                            wf28_a[:, p, :, oc * 128:(oc + 1) * 128],
                            ht8[p][:, :, cc],
                            start=(p == 0), stop=(p == 3), perf_mode=DR)
                    ot = sp.tile([128, 512], f32, tag="ot", bufs=4,
                                 name=f"ot{oc}_{s2}")
                    nc.vector.tensor_add(ot[:], cps[:], x2s[oc][:, cc])
                    oq = (nc.sync, nc.scalar)[oc % 2]
                    oq.dma_start(
                        outT_d.ap()[oc * 128:(oc + 1) * 128, cc], ot[:])

    nc.compile()
    return nc


def _get_program():
    global _COMPILED
    if _COMPILED is None:
        _COMPILED = _build_program()
    return _COMPILED


def kernel(x, wqkv, w_lin, b_lin, w_ff1, b_ff1, w_ff2, b_ff2):
    from concourse.bass_utils import run_bass_kernel_spmd

    x = np.asarray(x, np.float32)
    wqkv = np.asarray(wqkv, np.float32)
    Wq = wqkv[:, :D].astype(np.float64)
    Wk = wqkv[:, D:2 * D].astype(np.float64)
    Wv = wqkv[:, 2 * D:]

    def pmajor(a):
        """[1024, X] -> partition-major [128, 8*X] (chunk c = rows c*128+p)."""
        a = np.asarray(a)
        return np.ascontiguousarray(
            a.reshape(ND, 128, a.shape[1]).transpose(1, 0, 2)
        ).reshape(128, -1)

    wz = pmajor(((Wk @ Wq.T) / 2.0).astype(BF16))   # lhsT layout [d, a]
    wv = pmajor(Wv.astype(BF16))

    def dr_pack(w):
        """[D, D] -> fp8 DoubleRow layout [128, pair*2*D] (prescaled)."""
        w8 = np.asarray(np.asarray(w, np.float32) * FF_SCALE,
                        np.float32).astype(E4M3)
        return np.ascontiguousarray(
            w8.reshape(4, 2, 128, D).transpose(2, 0, 1, 3)
        ).reshape(128, 4 * 2 * D)

    wlin = (np.asarray(w_lin, np.float32) * FF_SCALE).astype(BF16)
    wff1 = dr_pack(w_ff1)
    wff2 = dr_pack(w_ff2)
    masks = {p: _build_masks(p) for p in (0, 1)}

    in_maps = []
    qcols_by_parity = {
        0: np.r_[0:512, 1536:2048],
        1: np.r_[512:1536],
    }
    b_lin = np.asarray(b_lin, np.float32)
    b_ff1 = np.asarray(b_ff1, np.float32)
    b_ff2 = np.asarray(b_ff2, np.float32)
    bf1 = np.ascontiguousarray(b_ff1.reshape(ND, 128).T)
    for c in range(NCORES):
        b, h = c // 2, c % 2
        xT32 = np.ascontiguousarray(x[b].T)               # [D, S] f32
        qcols = qcols_by_parity[h]
        qxT32 = np.ascontiguousarray(xT32[:, qcols])      # [D, 1024]
        in_maps.append({
            "xT": pmajor(xT32.astype(BF16)),
            "qxT": pmajor(qxT32.astype(BF16)),
            "xq": pmajor((qxT32 + b_lin[:, None]) * FF_SCALE),
            "wz": wz,
            "wv": wv,
            "wlin": wlin,
            "wff1": wff1,
            "wff2": wff2,
            "masks": masks[h],
            "bf1": bf1,
            "par": np.full((1, 1), h, np.uint32),
        })

    global _LAST_IN_MAPS
    _LAST_IN_MAPS = in_maps
    nc = _get_program()
    res = run_bass_kernel_spmd(nc, in_maps, core_ids=list(range(NCORES)))

    out = np.empty((B, S, D), np.float32)
    for c in range(NCORES):
        b, h = c // 2, c % 2
        ol = res.results[c]["outT"].T                     # [1024 s, D]
        if h == 0:
            out[b, 0:512] = ol[:512]
            out[b, 1536:2048] = ol[512:]
        else:
            out[b, 512:1536] = ol
    out *= 1.0 / FF_SCALE
    out += b_ff2[None, None, :]
    return out



# revision 32
# speedup vs baseline: 1.0569x; 1.0569x over previous
"""Trainium2 Bass kernel for nn_MultiHeadAttention_48395691492077.

Reference (B=4, S=2048, D=1024, single head, anti-causal triu mask):
    qkv = x @ wqkv; q,k,v = split(qkv)
    scores = triu(q @ k^T / sqrt(B));  masked softmax over keys t >= s
    x2  = softmax(scores) @ v @ w_lin + b_lin + x
    out = relu(x2 @ w_ff1 + b_ff1) @ w_ff2 + b_ff2 + x2

Sharding: 8 cores = 4 batches x 2 query-halves. Each core computes the
full-batch key/value projections (duplicated within a pair) and attention
for its own 1024 queries. The program is identical on all cores (SPMD);
per-core differences (which queries, which mask pattern) are carried in
the input data.

Device algebra (everything transposed so no on-chip transposes are needed):
    zT = wz^T.T @ xT   with wz = (Wk @ Wq^T)/2  (host-precomputed)
    scoresT[t,s] = sum_a zT[a,t] * xT[a,s]
    expT = exp(scoresT) * mask01               (no max-subtraction; |s|<=36)
    den[s] broadcast = ones[128,128].T @ expT (PE), rbs = 1/den (DVE)
    numerator^T[d,s] = v[t,d].T @ expT;  attn^T = numerator^T * rbs
    x2T = w_lin.T @ attn^T + (xT + b_lin);  hT = relu(w_ff1.T @ x2T + b_ff1)
    outT = w_ff2.T @ hT + x2T               (+ b_ff2 added on host)
Matmul inputs are bf16 (fp32 PSUM accumulation); residuals are fp32.
"""

import numpy as np
import ml_dtypes

B, S, D = 4, 2048, 1024
NCORES = 8
BF16 = ml_dtypes.bfloat16
E4M3 = ml_dtypes.float8_e4m3
FF_SCALE = 64.0  # fp8 weight prescale (keeps 0.02-sigma weights normal)

NT = S // 128            # 16 t-chunks
ND = D // 128            # 8 chunks of 128 along any D-sized dim

# global query-column starts of (sb0, sb1) per parity
SB_GLOBAL = {0: (0, 1536), 1: (512, 1024)}
# t-chunks each (parity, s-block) actually needs (branch-specialized)
SB_SLOTS = {
    0: {0: list(range(0, NT)), 1: list(range(12, NT))},
    1: {0: list(range(4, NT)), 1: list(range(8, NT))},
}


_COMPILED = None
_LAST_IN_MAPS = None


def _mask_order(parity: int):
    return [(sb, tc) for sb in (0, 1) for tc in SB_SLOTS[parity][sb]]


def _build_masks(parity: int) -> np.ndarray:
    """[20, 128, 512] bf16 multiplicative masks, one per processed block."""
    order = _mask_order(parity)
    m = np.zeros((len(order), 128, 512), np.float32)
    ii = np.arange(128)[:, None]
    jj = np.arange(512)[None, :]
    for k, (sb, tc) in enumerate(order):
        s0 = SB_GLOBAL[parity][sb]
        m[k] = ((128 * tc + ii) >= (s0 + jj)).astype(np.float32)
    return m.astype(BF16)


def _build_program():
    from contextlib import ExitStack
    import concourse.bacc as bacc
    import concourse.mybir as mybir
    import concourse.tile as tile

    f32 = mybir.dt.float32
    b16 = mybir.dt.bfloat16
    f8 = mybir.dt.float8e4
    AF = mybir.ActivationFunctionType
    DR = mybir.MatmulPerfMode.DoubleRow

    nc = bacc.Bacc("TRN2", target_bir_lowering=False, debug=False,
                   num_devices=NCORES)

    # xT/qxT/wz/wv arrive partition-major ([128, chunk, cols] flattened) so
    # every DMA is one contiguous multi-KB segment per partition
    xT_d = nc.dram_tensor("xT", [128, ND * S], b16, kind="ExternalInput")
    qxT_d = nc.dram_tensor("qxT", [128, ND * 1024], b16, kind="ExternalInput")
    xq_d = nc.dram_tensor("xq", [128, ND * 1024], f32, kind="ExternalInput")
    wz_d = nc.dram_tensor("wz", [128, ND * D], b16, kind="ExternalInput")
    wv_d = nc.dram_tensor("wv", [128, ND * D], b16, kind="ExternalInput")
    wlin_d = nc.dram_tensor("wlin", [D, D], b16, kind="ExternalInput")
    # FFN weights in fp8 DoubleRow layout [128, pair, 2, D] (prescaled)
    wff1_d = nc.dram_tensor("wff1", [128, ND * D], f8, kind="ExternalInput")
    wff2_d = nc.dram_tensor("wff2", [128, ND * D], f8, kind="ExternalInput")
    masks_d = nc.dram_tensor("masks", [20, 128, 512], b16, kind="ExternalInput")
    par_d = nc.dram_tensor("par", [1, 1], mybir.dt.uint32, kind="ExternalInput")
    bf1_d = nc.dram_tensor("bf1", [128, ND], f32, kind="ExternalInput")
    outT_d = nc.dram_tensor("outT", [D, 1024], f32, kind="ExternalOutput")

    with tile.TileContext(nc) as tc:
        es = ExitStack()
        with es:
            pp = es.enter_context(tc.tile_pool(name="persist", bufs=1))
            sp = es.enter_context(tc.tile_pool(name="stream", bufs=2))
            ps = es.enter_context(
                tc.tile_pool(name="ps", bufs=8, space="PSUM"))
            esB = es.enter_context(ExitStack())
            pb = esB.enter_context(tc.tile_pool(name="pB", bufs=1))
            esA = ExitStack()
            pa = esA.enter_context(tc.tile_pool(name="pA", bufs=1,
                                                side="right"))

            def psum():
                t = ps.tile([128, 512], f32, tag="mm", bufs=8, name="mmps")
                return t

            # ---- constants ----
            ones_sq = pp.tile([128, 128], b16, tag="ones_sq", bufs=1)
            nc.vector.memset(ones_sq[:], 1.0)
            # warm the PE HAM clock-gate while input DMAs are in flight
            wups = psum()
            NWARM = 40
            for i in range(NWARM):
                nc.tensor.matmul(wups[:, 0:128], ones_sq[:], ones_sq[:],
                                 start=(i == 0), stop=(i == NWARM - 1))

            # ---- input loads ----
            wv_a = pa.tile([128, ND, D], b16, tag="wv", bufs=1)
            xt_a = pa.tile([128, ND, S], b16, tag="xt", bufs=1)

            def xt_dma(eng, d, h):
                eng.dma_start(
                    xt_a[:, d, h * 1024:(h + 1) * 1024],
                    xT_d.ap()[:, d * S + h * 1024:d * S + (h + 1) * 1024])

            def wv_dma(eng, d, h):
                eng.dma_start(wv_a[:, d, h * 512:(h + 1) * 512],
                              wv_d.ap()[:, d * D + h * 512:
                                        d * D + (h + 1) * 512])

            # d-chunk supply races the d-major v loop below: per-d need is
            # wv half (.125MB) + xt t-half (.25MB) per 1.7us of compute.
            # sync starts immediately and feeds d=0..3, scalar (delayed by
            # its act-table preamble) d=4..7; the late-needed wv vb=1 halves
            # ride the slow gpsimd queue
            for d in range(5):
                wv_dma(nc.sync, d, 0)
                xt_dma(nc.sync, d, 0)
            for d in range(5, ND):
                xt_dma(nc.scalar, d, 0)
                wv_dma(nc.scalar, d, 0)
            for d in range(ND):
                wv_dma(nc.gpsimd, d, 1)
            for d in range(5):
                xt_dma(nc.sync, d, 1)
            for d in range(5, ND):
                xt_dma(nc.scalar, d, 1)
            wz_a = pa.tile([128, ND, D], b16, tag="wz", bufs=1)
            nc.sync.dma_start(
                wz_a[:], wz_d.ap().rearrange("p (c n) -> p c n", c=ND))
            qx_a = pb.tile([128, ND, 1024], b16, tag="qx", bufs=1)
            nc.scalar.dma_start(
                qx_a[:], qxT_d.ap().rearrange("p (c n) -> p c n", c=ND))
            bf1_t = pp.tile([128, ND], f32, tag="bf1", bufs=1)
            nc.sync.dma_start(bf1_t[:], bf1_d.ap())
            wv_t = [wv_a[:, d] for d in range(ND)]
            xt = [xt_a[:, d] for d in range(ND)]
            wz_t = [wz_a[:, d] for d in range(ND)]
            qx = [qx_a[:, d] for d in range(ND)]

            # ---- phase A: v [t,d] then zT [a,t] projections ----
            zt = [pb.tile([128, S], b16, tag=f"zt{m}", bufs=1, name=f"zt{m}")
                  for m in range(ND)]
            vt = [pb.tile([128, D], b16, tag=f"vt{t}", bufs=1, name=f"vt{t}")
                  for t in range(NT)]

            # quadrant (vb, t-half): 8 live psums, contraction (d) outermost
            # so the first quadrant consumes xt/wv chunk d as it lands
            for vb in range(2):
                for th in range(2):
                    trange = range(th * 8, th * 8 + 8)
                    vps = {t: psum() for t in trange}
                    for d in range(ND):
                        for t in trange:
                            nc.tensor.matmul(
                                vps[t][:],
                                xt[d][:, t * 128:(t + 1) * 128],
                                wv_t[d][:, vb * 512:(vb + 1) * 512],
                                start=(d == 0), stop=(d == ND - 1))
                    for t in trange:
                        nc.vector.tensor_copy(
                            vt[t][:, vb * 512:(vb + 1) * 512], vps[t][:])

            for m in range(ND):
                zps = [psum() for _ in range(4)]
                for d in range(ND):
                    for cb in range(4):
                        nc.tensor.matmul(
                            zps[cb][:],
                            wz_t[d][:, m * 128:(m + 1) * 128],
                            xt[d][:, cb * 512:(cb + 1) * 512],
                            start=(d == 0), stop=(d == ND - 1))
                for cb in range(4):
                    nc.vector.tensor_copy(
                        zt[m][:, cb * 512:(cb + 1) * 512], zps[cb][:])

            # ---- free phase-A inputs; right pool for attn + phase-C weights
            esA.close()
            pr = es.enter_context(tc.tile_pool(name="pAC", bufs=1,
                                               side="right"))
            wl_a = pr.tile([128, ND, D], b16, tag="wl", bufs=1)
            nc.sync.dma_start(wl_a[:], wlin_d.ap()
                              .rearrange("(c p) n -> p c n", p=128))
            wlin_t = [wl_a[:, d] for d in range(ND)]
            wf18_a = pr.tile([128, 4, 2, D], f8, tag="wf1", bufs=1)
            nc.sync.dma_start(
                wf18_a[:],
                wff1_d.ap().rearrange("p (a b n) -> p a b n", a=4, b=2))
            wf28_a = pr.tile([128, 4, 2, D], f8, tag="wf2", bufs=1)
            nc.sync.dma_start(
                wf28_a[:],
                wff2_d.ap().rearrange("p (a b n) -> p a b n", a=4, b=2))
            # prefetch the full residual (4MB f32) on the scalar queue during
            # phase B so phase C's out-DMAs get the sync queue to themselves
            xqf = pr.tile([128, ND, 1024], f32, tag="xqf", bufs=1)
            nc.scalar.dma_start(
                xqf[:], xq_d.ap().rearrange("p (c n) -> p c n", c=ND))

            attn = [pr.tile([128, 1024], b16, tag=f"at{d}", bufs=1,
                            name=f"at{d}") for d in range(ND)]

            def phase_b(parity):
                sb_slots = SB_SLOTS[parity]
                order = _mask_order(parity)
                sb0s = SB_GLOBAL[parity]

                def wclip(sb, tcn):
                    # query cols of block (sb,tcn) with any unmasked t form
                    # the prefix [0:w); cols beyond never touch PE or DVE
                    return min(512, 128 * tcn + 128 - sb0s[sb])

                # pass 1: scoresT -> exp -> mask, tcn descending so pass 2
                # (also descending: its start=True block must be full-width)
                # never waits on the tail of pass 1
                et = {}
                for tcn in reversed(range(NT)):
                    sbs = [sb for sb in (0, 1) if tcn in sb_slots[sb]]
                    scp = {sb: psum() for sb in sbs}
                    for a in range(ND):
                        for sb in sbs:
                            w = wclip(sb, tcn)
                            nc.tensor.matmul(
                                scp[sb][:, 0:w],
                                zt[a][:, tcn * 128:(tcn + 1) * 128],
                                qx[a][:, sb * 512:sb * 512 + w],
                                start=(a == 0), stop=(a == ND - 1))
                    for sb in sbs:
                        w = wclip(sb, tcn)
                        e = pb.tile([128, 512], b16, tag=f"et{sb}_{tcn}",
                                    bufs=1, name=f"et{parity}_{sb}_{tcn}")
                        et[(sb, tcn)] = e
                        nc.scalar.activation(e[:, 0:w], scp[sb][:, 0:w],
                                             AF.Exp)
                        if 128 * tcn - sb0s[sb] < 512:  # block not all-ones
                            kidx = order.index((sb, tcn))
                            mk = sp.tile([128, 512], b16, tag="mks", bufs=4,
                                         name=f"mk{parity}_{kidx}")
                            nc.sync.dma_start(mk[:, 0:w],
                                              masks_d.ap()[kidx][:, 0:w])
                            nc.vector.tensor_mul(e[:, 0:w], e[:, 0:w],
                                                 mk[:, 0:w])

                # pass 2: den (broadcast), recip, AV, normalize
                rbs = {}
                for sb in (0, 1):
                    slots = sorted(sb_slots[sb], reverse=True)
                    den_ps = psum()
                    for k, tcn in enumerate(slots):
                        w = wclip(sb, tcn)
                        nc.tensor.matmul(
                            den_ps[:, 0:w], ones_sq[:],
                            et[(sb, tcn)][:, 0:w],
                            start=(k == 0), stop=(k == len(slots) - 1))
                    r = sp.tile([128, 512], f32, tag="rbs", bufs=2,
                                name=f"rbs{parity}_{sb}")
                    nc.vector.reciprocal(r[:], den_ps[:])
                    rbs[sb] = r

                for dc in range(ND):
                    avp = {sb: psum() for sb in (0, 1)}
                    for sb in (0, 1):
                        slots = sorted(sb_slots[sb], reverse=True)
                        for k, tcn in enumerate(slots):
                            w = wclip(sb, tcn)
                            nc.tensor.matmul(
                                avp[sb][:, 0:w],
                                vt[tcn][:, dc * 128:(dc + 1) * 128],
                                et[(sb, tcn)][:, 0:w],
                                start=(k == 0), stop=(k == len(slots) - 1))
                    for sb in (0, 1):
                        nc.vector.tensor_mul(
                            attn[dc][:, sb * 512:(sb + 1) * 512],
                            avp[sb][:], rbs[sb][:])

            par_regs = nc.alloc_registers("par_regs")
            nc.regs_load(par_regs, par_d.ap()[0:1, 0:1])
            par = nc.snap(par_regs, donate=True, min_val=0, max_val=1)
            with tc.If(par < 1) as cmp:
                phase_b(0)
            with cmp.Else():
                phase_b(1)

            # ---- free pB (zt/vt/qx/et); left pool for phase-C tiles ----
            esB.close()
            esC = es.enter_context(ExitStack())
            pc = esC.enter_context(tc.tile_pool(name="pC", bufs=1))

            x2s = [pc.tile([128, 1024], f32, tag=f"x2s{d}", bufs=1,
                           name=f"x2s{d}") for d in range(ND)]
            x28 = [pc.tile([128, 2, 1024], f8, tag=f"x28{p}", bufs=1,
                           name=f"x28{p}") for p in range(4)]
            ht8 = [pc.tile([128, 2, 1024], f8, tag=f"ht8{p}", bufs=1,
                           name=f"ht8{p}") for p in range(4)]

            # phase C per s2-half: the first half's outputs start draining
            # while the second half computes (out-DMA would otherwise bound
            # the kernel tail)
            for s2 in range(2):
                cc = slice(s2 * 512, (s2 + 1) * 512)
                for oc in range(ND):
                    cps = psum()
                    for d in range(ND):
                        nc.tensor.matmul(
                            cps[:],
                            wlin_t[d][:, oc * 128:(oc + 1) * 128],
                            attn[d][:, cc],
                            start=(d == 0), stop=(d == ND - 1))
                    nc.vector.tensor_add(x2s[oc][:, cc], cps[:],
                                         xqf[:, oc, cc])
                    nc.scalar.activation(x28[oc // 2][:, oc % 2, cc],
                                         x2s[oc][:, cc], AF.Copy,
                                         scale=1.0 / FF_SCALE)

                for fc in range(ND):
                    cps = psum()
                    for p in range(4):
                        nc.tensor.matmul(
                            cps[:],
                            wf18_a[:, p, :, fc * 128:(fc + 1) * 128],
                            x28[p][:, :, cc],
                            start=(p == 0), stop=(p == 3), perf_mode=DR)
                    nc.scalar.activation(ht8[fc // 2][:, fc % 2, cc], cps[:],
                                         AF.Relu, bias=bf1_t[:, fc:fc + 1],
                                         scale=1.0 / FF_SCALE)

                for oc in range(ND):
                    cps = psum()
                    for p in range(4):
                        nc.tensor.matmul(
                            cps[:],
                            wf28_a[:, p, :, oc * 128:(oc + 1) * 128],
                            ht8[p][:, :, cc],
                            start=(p == 0), stop=(p == 3), perf_mode=DR)
                    ot = sp.tile([128, 512], f32, tag="ot", bufs=4,
                                 name=f"ot{oc}_{s2}")
                    nc.vector.tensor_add(ot[:], cps[:], x2s[oc][:, cc])
                    oq = (nc.sync, nc.scalar)[oc % 2]
                    oq.dma_start(
                        outT_d.ap()[oc * 128:(oc + 1) * 128, cc], ot[:])

    nc.compile()
    return nc


def _get_program():
    global _COMPILED
    if _COMPILED is None:
        _COMPILED = _build_program()
    return _COMPILED


def kernel(x, wqkv, w_lin, b_lin, w_ff1, b_ff1, w_ff2, b_ff2):
    from concourse.bass_utils import run_bass_kernel_spmd

    x = np.asarray(x, np.float32)
    wqkv = np.asarray(wqkv, np.float32)
    Wq = wqkv[:, :D].astype(np.float64)
    Wk = wqkv[:, D:2 * D].astype(np.float64)
    Wv = wqkv[:, 2 * D:]

    def pmajor(a):
        """[1024, X] -> partition-major [128, 8*X] (chunk c = rows c*128+p)."""
        a = np.asarray(a)
        return np.ascontiguousarray(
            a.reshape(ND, 128, a.shape[1]).transpose(1, 0, 2)
        ).reshape(128, -1)

    wz = pmajor(((Wk @ Wq.T) / 2.0).astype(BF16))   # lhsT layout [d, a]
    wv = pmajor(Wv.astype(BF16))

    def dr_pack(w):
        """[D, D] -> fp8 DoubleRow layout [128, pair*2*D] (prescaled)."""
        w8 = np.asarray(np.asarray(w, np.float32) * FF_SCALE,
                        np.float32).astype(E4M3)
        return np.ascontiguousarray(
            w8.reshape(4, 2, 128, D).transpose(2, 0, 1, 3)
        ).reshape(128, 4 * 2 * D)

    wlin = (np.asarray(w_lin, np.float32) * FF_SCALE).astype(BF16)
    wff1 = dr_pack(w_ff1)
    wff2 = dr_pack(w_ff2)
    masks = {p: _build_masks(p) for p in (0, 1)}

    in_maps = []
    qcols_by_parity = {
        0: np.r_[0:512, 1536:2048],
        1: np.r_[512:1536],
    }
    b_lin = np.asarray(b_lin, np.float32)
    b_ff1 = np.asarray(b_ff1, np.float32)
    b_ff2 = np.asarray(b_ff2, np.float32)
    bf1 = np.ascontiguousarray(b_ff1.reshape(ND, 128).T)
    for c in range(NCORES):
        b, h = c // 2, c % 2
        xT32 = np.ascontiguousarray(x[b].T)               # [D, S] f32
        qcols = qcols_by_parity[h]
        qxT32 = np.ascontiguousarray(xT32[:, qcols])      # [D, 1024]
        in_maps.append({
            "xT": pmajor(xT32.astype(BF16)),
            "qxT": pmajor(qxT32.astype(BF16)),
            "xq": pmajor((qxT32 + b_lin[:, None]) * FF_SCALE),
            "wz": wz,
            "wv": wv,
            "wlin": wlin,
            "wff1": wff1,
            "wff2": wff2,
            "masks": masks[h],
            "bf1": bf1,
            "par": np.full((1, 1), h, np.uint32),
        })

    global _LAST_IN_MAPS
    _LAST_IN_MAPS = in_maps
    nc = _get_program()
    res = run_bass_kernel_spmd(nc, in_maps, core_ids=list(range(NCORES)))

    out = np.empty((B, S, D), np.float32)
    for c in range(NCORES):
        b, h = c // 2, c % 2
        ol = res.results[c]["outT"].T                     # [1024 s, D]
        if h == 0:
            out[b, 0:512] = ol[:512]
            out[b, 1536:2048] = ol[512:]
        else:
            out[b, 512:1536] = ol
    out *= 1.0 / FF_SCALE
    out += b_ff2[None, None, :]
    return out

